# revision 1
# baseline (speedup 1.0000x reference)
"""Mass-spring substep integrator on 8 Trainium2 NeuronCores.

Topology ("quad" edge-sharded, batch-replicated): every core carries all
4 batches; node ranks are dealt round-robin onto the 8 cores and each
core owns the incidences of its ranks, so the per-node force sums it
computes are complete — no cross-core force reduction. Once per substep
the integrated positions of each core's slab are AllGathered into a full
position table that next substep's gathers read.

Per-core data layout:
  - nodes are relabeled on the host: sorted by incidence count, grouped
    into "global ranks" of 128 (one node per SBUF partition), and global
    rank r = 8j + h is owned by core h with core-local rank j. State is
    node-major: pos[p, ((j*4 + b)*3 + c)] (m = 12 floats per node).
  - the owned directed incidences are laid out in a [128, J] slot grid
    grouped by owner rank, with a degree-profile template (segment size
    per rank j) SHARED across cores, so owner-side broadcast / segmented
    reduction are plain strided vector ops.
  - partner positions (48-byte per-node records covering all 4 batches)
    for a whole chunk of slot columns are fetched with ONE batched
    indirect DMA per chunk from the AllGathered table.

All index tables are precomputed on the host from the (static) edge
list; outputs are un-permuted back to the original node order on the
host after the device run.
"""

import numpy as np

import concourse.bass as bass
import concourse.mybir as mybir
import concourse.tile as tile
from concourse.bass_utils import run_bass_kernel_spmd

# Problem constants (must match the reference)
B, NV, NE, SUBSTEPS = 4, 100000, 400000, 10
DT = 0.01
K_SPRING = 1000.0
MASS = 1.0
DAMP = 0.999
ACT_SCALE = 0.1
EPS = 1e-6
GRAVITY_Y = -9.8

P = 128           # SBUF partitions
NCORE = 8
# Cumulative slot fractions per chunk: equal-ish front, shrinking tail chunks
# so the post-train compute tail on the critical path stays short.
CHUNK_FRACS = [0.14, 0.28, 0.42, 0.56, 0.69, 0.81, 0.90, 0.96, 0.99, 1.0]
NCHUNK = len(CHUNK_FRACS)
M = B * 3         # floats per node record
PAD_REST = float(np.sqrt(EPS))  # rest length that zeroes force on d=0 pad slots


# ---------------------------------------------------------------------------
# walrus workaround: this toolchain accepts only ONE sync-wait per
# instruction; split extra waits onto fresh same-engine NOPs.
# ---------------------------------------------------------------------------
_ctr = [0]


def _split_multi_waits(nc):
    for f in nc.m.functions:
        for b in f.blocks:
            old = b.instructions
            new = []
            changed = False
            for inst in old:
                si = inst.sync_info
                if si is not None and si.on_wait is not None and len(si.on_wait) > 1:
                    waits = list(si.on_wait)
                    for w in waits[:-1]:
                        _ctr[0] += 1
                        nop = mybir.InstNoOp(
                            name=f"SPLITW-{_ctr[0]}",
                            engine=inst.engine,
                            ins=[], outs=[],
                            sync_info=mybir.SyncInfo(on_wait=[w], on_update=[]),
                        )
                        new.append(nop)
                    si.on_wait = waits[-1:]
                    changed = True
                new.append(inst)
            if changed:
                b.instructions = new


class _TileContext(tile.TileContext):
    def __exit__(self, *args):
        r = super().__exit__(*args)
        if args[0] is None:
            _split_multi_waits(self.nc)
        return r


# ---------------------------------------------------------------------------
# Host-side plan construction (static, depends only on the edge list)
# ---------------------------------------------------------------------------
class Plan:
    pass


def build_plan(edges, nv, ne):
    """Relabel nodes by degree, deal ranks over cores, build the shared
    degree-profile slot template and per-core slot index tables."""
    nvp = -(-nv // P)
    nvp = -(-nvp // NCORE) * NCORE        # multiple of NCORE
    nvtot = nvp * P
    njc = nvp // NCORE                     # core-local ranks ("j")

    i_idx = edges[:, 0].astype(np.int64)
    j_idx = edges[:, 1].astype(np.int64)

    deg = np.bincount(edges.ravel(), minlength=nv).astype(np.int64)
    deg_pad = np.concatenate([deg, np.zeros(nvtot - nv, np.int64)])

    # global rank r <- the r-th group of 128 nodes in degree-sorted order
    order = np.argsort(-deg_pad, kind="stable")
    grid_nodes = order.reshape(nvp, P)          # [r, p]
    r_of = np.zeros(nvtot, np.int32)
    p_of = np.zeros(nvtot, np.int32)
    p_of[grid_nodes.ravel()] = np.tile(np.arange(P, dtype=np.int32), nvp)
    r_of[grid_nodes.ravel()] = np.repeat(np.arange(nvp, dtype=np.int32), P)

    dr = deg_pad[grid_nodes[:, 0]]              # max degree per global rank
    # shared per-core template: d_j = max over cores = dr[8j] (sorted desc)
    dj = dr[0::NCORE].copy()                    # [njc]
    seg_start = np.zeros(njc + 1, np.int64)
    seg_start[1:] = np.cumsum(dj)
    J = int(seg_start[-1])

    # degree classes: runs of equal dj with dj >= 1
    classes = []
    k = 0
    while k < njc:
        d = int(dj[k])
        k2 = k
        while k2 < njc and dj[k2] == d:
            k2 += 1
        if d >= 1:
            classes.append((k, k2, d))
        k = k2

    # chunks: split the slot range at rank boundaries; every rank belongs to
    # exactly one chunk (integration runs per chunk).
    targets = [int(round(J * f)) for f in CHUNK_FRACS]
    bounds_k = [0]
    for t in targets[:-1]:
        kb = int(np.searchsorted(seg_start, t, side="left"))
        kb = max(min(kb, njc), bounds_k[-1])
        bounds_k.append(kb)
    bounds_k.append(njc)
    chunks = []
    for ci in range(NCHUNK):
        klo, khi = bounds_k[ci], bounds_k[ci + 1]
        if khi == klo:
            continue
        lo, hi = int(seg_start[klo]), int(seg_start[khi])
        pieces = []
        for (ka, kb, d) in classes:
            a, b2 = max(ka, klo), min(kb, khi)
            if a < b2:
                pieces.append((a, b2, d))
        chunks.append(dict(klo=klo, khi=khi, lo=lo, hi=hi, pieces=pieces))

    # gather-table row of a node: h = r % 8, j = r // 8 -> (h*128 + p)*njc + j
    h_of = (r_of % NCORE).astype(np.int64)
    j_of = (r_of // NCORE).astype(np.int64)
    row_of = (h_of * P + p_of) * njc + j_of     # [nvtot]

    # per-core slot tables
    iu = np.concatenate([i_idx, j_idx])
    iv = np.concatenate([j_idx, i_idx])
    ee = np.concatenate([np.arange(ne, dtype=np.int64)] * 2)
    hu = h_of[iu]
    ju = j_of[iu]
    pu = p_of[iu].astype(np.int64)

    rank_of_slot = np.repeat(np.arange(njc, dtype=np.int64), dj)  # [J]

    pidx = np.zeros((NCORE, P, J), np.int32)
    eidx = np.full((NCORE, P, J), -1, np.int64)
    for h in range(NCORE):
        sel = hu == h
        pv, jv, vv, ev = pu[sel], ju[sel], iv[sel], ee[sel]
        so = np.lexsort((ev, jv, pv))
        pv, jv, vv, ev = pv[so], jv[so], vv[so], ev[so]
        grp = pv * njc + jv
        uniq, first = np.unique(grp, return_index=True)
        within = np.arange(len(grp)) - np.repeat(
            first, np.diff(np.append(first, len(grp))))
        jpos = seg_start[jv] + within
        # pads point at the owner itself (d = 0)
        own_rows = ((h * P + np.arange(P, dtype=np.int64))[:, None] * njc
                    + rank_of_slot[None, :])
        pidx[h] = own_rows.astype(np.int32)
        pidx[h, pv, jpos] = row_of[vv].astype(np.int32)
        eidx[h, pv, jpos] = ev

    plan = Plan()
    plan.nv, plan.ne = nv, ne
    plan.nvp, plan.nvtot, plan.njc, plan.J = nvp, nvtot, njc, J
    plan.classes = classes
    plan.chunks = chunks
    plan.seg_start = seg_start
    plan.pidx = pidx
    plan.eidx = eidx
    plan.grid_nodes = grid_nodes
    plan.cw_max = max(c["hi"] - c["lo"] for c in chunks)
    return plan


def host_core_inputs(plan, h, input_pos, input_vel, input_action, rest_len):
    """Per-core initial state [P, njc*M], slot index table and -K*rest_eff."""
    njc, nv = plan.njc, plan.nv
    sub = plan.grid_nodes[h::NCORE]           # [njc, P] node ids
    valid = sub < nv
    gp = np.clip(sub, 0, nv - 1)
    ps = input_pos[:, gp].copy()              # [B, j, p, 3]
    vs = input_vel[:, gp].copy()
    ps[:, ~valid] = 0.0
    vs[:, ~valid] = 0.0
    # -> [p, j, b, c]
    pos = ps.transpose(2, 1, 0, 3).reshape(P, njc * M)
    vel = vs.transpose(2, 1, 0, 3).reshape(P, njc * M)

    e = plan.eidx[h]
    pad = e < 0
    ec = np.clip(e, 0, plan.ne - 1)
    rest_eff = rest_len[ec][None] * (
        1.0 + ACT_SCALE * np.tanh(input_action[:, ec]))   # [B, P, J]
    nkr = (-K_SPRING * rest_eff).astype(np.float32)
    nkr[:, pad] = -K_SPRING * PAD_REST
    nkr = np.ascontiguousarray(nkr.transpose(1, 2, 0).reshape(P, plan.J * B))
    return {
        "pos0": np.ascontiguousarray(pos, dtype=np.float32),
        "vel0": np.ascontiguousarray(vel, dtype=np.float32),
        "pidx": np.ascontiguousarray(plan.pidx[h]),
        "nkr": nkr,
    }


# ---------------------------------------------------------------------------
# Device kernel
# ---------------------------------------------------------------------------
def _ins_bcast(ap, pos_idx, count):
    dims = [list(x) for x in ap.ap]
    dims.insert(pos_idx, [0, count])
    return bass.AP(ap.tensor, ap.offset, dims)


DEBUG_DUMP = False


def _integrate_piece(nc, ch, plan, pos, pos16, vel, fsum, grav_t, cc_in,
                     last=False):
    """Symplectic-Euler update for one chunk's rank range:
    t = fsum*DT + vel ; t_y += DT*G ; vel = DAMP*t ; pos += DT*vel;
    then refresh the fp16 slab and its share of next substep's cc_in."""
    a, b2 = ch["klo"] * M, ch["khi"] * M
    fp = fsum[:, a:b2]
    vp = vel[:, a:b2]
    pp = pos[:, a:b2]
    p16 = pos16[:, a:b2]
    nc.vector.scalar_tensor_tensor(
        out=fp, in0=fp, scalar=float(DT / MASS), in1=vp,
        op0=mybir.AluOpType.mult, op1=mybir.AluOpType.add)
    yv = fp.rearrange("p (q c) -> p q c", c=3)[:, :, 1:2]
    nc.scalar.activation(
        yv, yv, mybir.ActivationFunctionType.Identity, bias=grav_t[:])
    # pos += DT*DAMP*t (DVE) runs in parallel with vel = DAMP*t (ACT)
    nc.vector.scalar_tensor_tensor(
        out=pp, in0=fp, scalar=float(DT * DAMP), in1=pp,
        op0=mybir.AluOpType.mult, op1=mybir.AluOpType.add)
    nc.scalar.activation(
        vp, fp, mybir.ActivationFunctionType.Copy, scale=float(DAMP))
    if not last:
        nc.scalar.activation(p16, pp, mybir.ActivationFunctionType.Copy)
        nc.sync.dma_start(cc_in[:, a:b2], p16)


def build_bass(plan, substeps):
    njc, J, nvtot = plan.njc, plan.J, plan.nvtot
    npm = njc * M
    cwm = plan.cw_max
    f32 = mybir.dt.float32

    nc = bass.Bass(num_devices=NCORE)
    pos0 = nc.dram_tensor("pos0", [P, npm], f32, kind="ExternalInput")
    vel0 = nc.dram_tensor("vel0", [P, npm], f32, kind="ExternalInput")
    pidx = nc.dram_tensor("pidx", [P, J], mybir.dt.int32, kind="ExternalInput")
    nkr_in = nc.dram_tensor("nkr", [P, J * B], f32, kind="ExternalInput")
    tab0 = nc.dram_tensor("tab0", [NCORE, P, npm], mybir.dt.float16,
                          kind="ExternalInput")

    opos = nc.dram_tensor("opos", [substeps + 1, P, npm], f32,
                          kind="ExternalOutput")
    ovel = nc.dram_tensor("ovel", [substeps + 1, P, npm], f32,
                          kind="ExternalOutput")

    f16 = mybir.dt.float16
    cc_in = nc.dram_tensor("cc_in", [P, npm], f16, kind="Internal")
    cc_out = nc.dram_tensor("cc_out", [NCORE, P, npm], f16, kind="Internal")
    if DEBUG_DUMP:
        dbg_tab = nc.dram_tensor("dbg_tab", [NCORE, P, npm], f16,
                                 kind="ExternalOutput")
        dbg_rem = nc.dram_tensor("dbg_rem", [P, J * M], f16,
                                 kind="ExternalOutput")
        dbg_d = nc.dram_tensor("dbg_d", [P, J * M], f16,
                               kind="ExternalOutput")
        dbg_fsum = nc.dram_tensor("dbg_fsum", [P, npm], f32,
                                  kind="ExternalOutput")
    # gather-table view: row (h*128+p)*njc + j holds that node's 12 floats
    tab = cc_out[:].rearrange("h p (j m) -> (h p j) m", m=M)
    tab0_v = tab0[:].rearrange("h p (j m) -> (h p j) m", m=M)

    with _TileContext(nc) as tc:
        with tc.tile_pool(name="state", bufs=1) as pool:
            pos = pool.tile([P, npm], f32, name="pos")
            pos16 = pool.tile([P, npm], f16, name="pos16")
            vel = pool.tile([P, npm], f32, name="vel")
            fsum = pool.tile([P, npm], f32, name="fsum")
            pidx_sb = pool.tile([P, J], mybir.dt.int32, name="pidx_sb")
            nkr_sb = pool.tile([P, J * B], f32, name="nkr_sb")
            eps_t = pool.tile([P, 1], f32, name="eps_t")
            grav_t = pool.tile([P, 1], f32, name="grav_t")
            rem = [pool.tile([P, cwm * M], f16, name=f"rem{b}")
                   for b in range(2)]
            sq = [pool.tile([P, cwm * M], f32, name=f"sq{b}")
                  for b in range(2)]
            s2 = [pool.tile([P, cwm * B], f32, name=f"s2{b}")
                  for b in range(2)]
            inv = [pool.tile([P, cwm * B], f32, name=f"inv{b}")
                   for b in range(2)]

            pos_km = pos[:].rearrange("p (k m) -> p k m", m=M)
            pos16_km = pos16[:].rearrange("p (k m) -> p k m", m=M)
            fsum_km = fsum[:].rearrange("p (k m) -> p k m", m=M)

            # ---- one-time setup ----
            nc.vector.memset(eps_t[:], float(EPS))
            nc.vector.memset(grav_t[:], float(GRAVITY_Y * DT))
            nc.vector.memset(fsum[:], 0.0)
            nc.sync.dma_start(pos[:], pos0[:])
            nc.sync.dma_start(vel[:], vel0[:])
            nc.scalar.activation(
                pos16[:], pos[:], mybir.ActivationFunctionType.Copy)
            nc.sync.dma_start(pidx_sb[:], pidx[:])
            nc.sync.dma_start(nkr_sb[:], nkr_in[:])
            nc.sync.dma_start(opos[0], pos[:])
            nc.sync.dma_start(ovel[0], vel[:])

            # ---- substeps (statically unrolled) ----
            for s in range(substeps):
                # 1) AllGather the table directly from the SBUF fp16 slab
                # (refreshed piecewise during the previous substep's
                # integration). Substep 0 reads the host-built table instead.
                if s == 0:
                    tab_s = tab0_v
                else:
                    nc.gpsimd.collective_compute(
                        "AllGather", mybir.AluOpType.bypass,
                        replica_groups=[list(range(NCORE))],
                        ins=[cc_in[:]], outs=[cc_out[:]],
                    )
                    tab_s = tab
                if DEBUG_DUMP and s == 1:
                    nc.sync.dma_start(dbg_tab[:], cc_out[:])

                for ci, ch in enumerate(plan.chunks):
                    bi = ci % 2
                    lo, hi = ch["lo"], ch["hi"]
                    cw = hi - lo
                    if cw == 0:
                        _integrate_piece(nc, ch, plan, pos, pos16, vel, fsum,
                                         grav_t, cc_in,
                                         last=(s == substeps - 1))
                        continue
                    remc = rem[bi][:, :cw * M]
                    sqc = sq[bi][:, :cw * M]
                    s2c = s2[bi][:, :cw * B]
                    invc = inv[bi][:, :cw * B]
                    rem_jm = remc.rearrange("p (j m) -> p j m", m=M)
                    rem_jbc = remc.rearrange("p (j b c) -> p j b c", b=B, c=3)

                    # 2) gather partner records (one indirect DMA per slot
                    # column: the DGE consumes one offset per partition)
                    for j in range(lo, hi):
                        nc.gpsimd.indirect_dma_start(
                            out=rem[bi][:, (j - lo) * M:(j - lo + 1) * M],
                            out_offset=None,
                            in_=tab_s,
                            in_offset=bass.IndirectOffsetOnAxis(
                                ap=pidx_sb[:, j:j + 1], axis=0),
                        )
                    if DEBUG_DUMP and s == 0:
                        nc.sync.dma_start(
                            dbg_rem[:, lo * M:hi * M], remc)
                    # 3) d = partner - owner (per degree-class piece, fp16)
                    for (ka, kb, d) in ch["pieces"]:
                        s0 = int(plan.seg_start[ka]) - lo
                        nk = kb - ka
                        dst = rem_jm[:, s0:s0 + nk * d, :].rearrange(
                            "p (n dd) m -> p n dd m", dd=d)
                        src = _ins_bcast(pos16_km[:, ka:kb, :], 2, d)
                        nc.vector.tensor_tensor(
                            out=dst, in0=dst, in1=src,
                            op=mybir.AluOpType.subtract)
                    if DEBUG_DUMP and s == 0:
                        nc.sync.dma_start(dbg_d[:, lo * M:hi * M], remc)
                    # 4) sq = d*d, s2 = sum_c sq (both DVE: keeps the tail
                    # chain on one engine, saving cross-engine sem hops)
                    nc.vector.tensor_tensor(
                        out=sqc, in0=remc, in1=remc,
                        op=mybir.AluOpType.mult)
                    nc.vector.tensor_reduce(
                        out=s2c.rearrange("p (j b) -> p j b", b=B),
                        in_=sqc.rearrange("p (j b c) -> p j b c", b=B, c=3),
                        axis=mybir.AxisListType.X, op=mybir.AluOpType.add)
                    # 5) len = sqrt(s2 + eps) (ACT, in place)
                    nc.scalar.activation(
                        s2c, s2c, mybir.ActivationFunctionType.Sqrt,
                        bias=eps_t[:])
                    # 6) inv = 1/len ; t = nkr*inv  (in place in inv)
                    nc.vector.reciprocal(invc, s2c)
                    nc.vector.tensor_tensor(
                        out=invc, in0=nkr_sb[:, lo * B:hi * B], in1=invc,
                        op=mybir.AluOpType.mult)
                    # 7) f = (t + K) * d   (into sq, fp32; d is fp16)
                    sq_jm = sqc.rearrange("p (j m) -> p j m", m=M)
                    sq_jbc = sqc.rearrange("p (j b c) -> p j b c", b=B, c=3)
                    tb = _ins_bcast(
                        invc.rearrange("p (j b) -> p j b", b=B), 3, 3)
                    nc.vector.scalar_tensor_tensor(
                        out=sq_jbc, in0=tb, scalar=float(K_SPRING),
                        in1=rem_jbc, op0=mybir.AluOpType.add,
                        op1=mybir.AluOpType.mult)
                    # 8) segmented reduce -> fsum
                    for (ka, kb, d) in ch["pieces"]:
                        s0 = int(plan.seg_start[ka]) - lo
                        nk = kb - ka
                        src = sq_jm[:, s0:s0 + nk * d, :].rearrange(
                            "p (n dd) m -> p n m dd", dd=d)
                        dst = fsum_km[:, ka:kb, :]
                        nc.vector.tensor_reduce(
                            out=dst, in_=src, axis=mybir.AxisListType.X,
                            op=mybir.AluOpType.add)
                    # 9) integrate this chunk's ranks (their force sums are
                    # complete) while later chunks still gather
                    _integrate_piece(nc, ch, plan, pos, pos16, vel, fsum,
                                     grav_t, cc_in,
                                     last=(s == substeps - 1))

                if DEBUG_DUMP and s == 0:
                    nc.sync.dma_start(dbg_fsum[:], fsum[:])
                # 10) trajectory
                nc.sync.dma_start(opos[s + 1], pos[:])
                nc.sync.dma_start(ovel[s + 1], vel[:])

    return nc


# ---------------------------------------------------------------------------
# Entry point
# ---------------------------------------------------------------------------
_cache = {}


def _get_plan_and_bass(edges, nv, ne, substeps):
    kh = (hash(edges.tobytes()), nv, ne, substeps)
    if kh not in _cache:
        plan = build_plan(edges, nv, ne)
        nc = build_bass(plan, substeps)
        _cache[kh] = (plan, nc)
    return _cache[kh]


def kernel(input_action, input_pos, input_vel, rest_len, edges):
    input_action = np.asarray(input_action, np.float32)
    input_pos = np.asarray(input_pos, np.float32)
    input_vel = np.asarray(input_vel, np.float32)
    rest_len = np.asarray(rest_len, np.float32)
    edges = np.asarray(edges, np.int32)

    nb, nv, _ = input_pos.shape
    ne = edges.shape[0]
    plan, nc = _get_plan_and_bass(edges, nv, ne, SUBSTEPS)

    in_maps = [
        host_core_inputs(plan, h, input_pos, input_vel, input_action, rest_len)
        for h in range(NCORE)
    ]
    tab0 = np.ascontiguousarray(
        np.stack([im["pos0"] for im in in_maps]).astype(np.float16))
    for im in in_maps:
        im["tab0"] = tab0
    res = run_bass_kernel_spmd(nc, in_maps, core_ids=list(range(NCORE)))

    s1 = SUBSTEPS + 1
    out_pos = np.empty((nb, s1, nv, 3), np.float32)
    out_vel = np.empty((nb, s1, nv, 3), np.float32)
    for h in range(NCORE):
        sub = plan.grid_nodes[h::NCORE]       # [njc, P]
        mask = sub < nv
        jj, pp = np.nonzero(mask)
        ids = sub[jj, pp]
        r = res.results[h]
        tp = r["opos"].reshape(s1, P, plan.njc, nb, 3)
        tv = r["ovel"].reshape(s1, P, plan.njc, nb, 3)
        for b in range(nb):
            out_pos[b][:, ids] = tp[:, pp, jj, b]
            out_vel[b][:, ids] = tv[:, pp, jj, b]
    return out_pos, out_vel

